# revision 30
# baseline (speedup 1.0000x reference)
"""Distributed MHA kernel for one TRN2 chip (8 NeuronCores), Bass/Tile.

Problem: B=4, S=2048, D=1024, H=16 full multi-head attention
(qkv proj -> scaled dot product softmax attention -> o proj).

Sharding (no collectives): core c handles batch b=c//2 and query-token
half c%2 (1024 query tokens).  Each core recomputes K/V projections for
the full 2048 tokens of its batch (+25% PE work, zero cross-core sync).
The host permutes x[b] so the core's query tokens come first; softmax
over keys is permutation invariant, so K/V token order doesn't matter.

On-chip dataflow (per core), bf16 operands, fp32 PSUM:
  logits^T [ktok, q] = K_h^T.T @ Q_h^T  (contract hd=64, head pairs
      packed at partition offsets 0/64)
  P^T = exp(0.125 * logits^T)           (ACT; logits ~ N(0,1), safe)
  PV in [q, d] layout: out[q128, 65] += P^T[ktok, q128].T @ V_aug[ktok, 65]
      (V_aug has a ones column -> col 64 accumulates the softmax
      denominator per q partition; 65-col matmuls instead of 512-col
      halve the PE cycles of the PV stage)
  normalize: per-partition reciprocal+multiply (DVE)
  vals arrives [q, d]-transposed for o-proj -> PE transpose (identity
      matmul) back to [d, q], evacuate to SBUF
  o proj: out[tok, e] = vals^T.T @ o_w^T, contraction split in two
      halves so the first half runs mid-attention; bf16 output store

Scheduling (the whole point): the attention inner loop alone is
ACT(exp)-paced at ~1038ns/iter vs ~640ns of PE work, so all K/V/Q/O
projection matmuls are fed through pull-driven "filler" streams
interleaved between attention matmuls; the PE stays ~95% busy and is
the binding engine end to end.  Stream markers guarantee V/K/Q of
pair p are emitted before attention of pair p; the two q-halves'
attention is interleaved at pair granularity to spread filler demand;
PV consumes exp output with a two-iteration lag to hide the ACT
pipeline+semaphore latency; each pair's transpose+evac is deferred
into the next pair's second iteration so the PE never waits on the
DVE normalize.
"""

import numpy as np

_NC_CACHE = {}


def _build_nc(S, D, H, SQ, use_bf16=True):
    import concourse.bass as bass
    import concourse.mybir as mybir
    import concourse.tile as tile
    from concourse import bacc
    from concourse.bass import ts
    from concourse.masks import make_identity

    f32 = mybir.dt.float32
    cdt = mybir.dt.bfloat16 if use_bf16 else f32
    Exp = mybir.ActivationFunctionType.Exp
    add = mybir.AluOpType.add
    mult = mybir.AluOpType.mult

    P = 128
    hd = D // H            # 64 head dim
    hd1 = hd + 1           # 65: V block + ones column
    ND = D // P            # 8 din/dout chunks
    NT = S // 512          # 4 tok512 chunks
    NQ = SQ // 512         # 2 q512 chunks
    NK = S // P            # 16 k-token chunks
    NPAIR = H // 2         # 8 head pairs (= dout chunks of K/Q)
    scale = 1.0 / float(np.sqrt(hd))

    nc = bacc.Bacc(trn_type="TRN2", debug=False)

    xT = nc.declare_dram_parameter("xT", [D, S], cdt, isOutput=False)
    wqT = nc.declare_dram_parameter("wqT", [D, D], cdt, isOutput=False)
    wkT = nc.declare_dram_parameter("wkT", [D, D], cdt, isOutput=False)
    wvT = nc.declare_dram_parameter("wvT", [D, D], cdt, isOutput=False)
    owT = nc.declare_dram_parameter("owT", [D, D], cdt, isOutput=False)
    bq = nc.declare_dram_parameter("bq", [D], f32, isOutput=False)
    bk = nc.declare_dram_parameter("bk", [D], f32, isOutput=False)
    bv = nc.declare_dram_parameter("bv", [D], f32, isOutput=False)
    bo = nc.declare_dram_parameter("bo", [D], f32, isOutput=False)
    # bf16 output halves the store traffic on the serialized DMA device
    # (host converts back to f32); costs ~0.2% relative error
    out = nc.declare_dram_parameter("out", [SQ, D], cdt, isOutput=True)

    xT_r = xT.ap().rearrange("(c p) s -> p c s", p=P)
    wqT_r = wqT.ap().rearrange("(c p) e -> p c e", p=P)
    wkT_r = wkT.ap().rearrange("(c p) e -> p c e", p=P)
    wvT_r = wvT.ap().rearrange("(c p) e -> p c e", p=P)
    owT_r = owT.ap().rearrange("(c p) e -> p c e", p=P)

    def mm(ps, lhsT, rhs, start, stop):
        nc.tensor.matmul(ps, lhsT, rhs, start=start, stop=stop)

    class Stream:
        """Pull-driven instruction-emission stream with markers.

        Units are dicts {dma: fn|None, comp: genfn, mark: key|None}; the
        comp generator yields approximate PE-cycle counts after each
        chunk of emitted instructions.  DMAs are emitted one unit ahead
        of compute so weights prefetch.
        """

        LOOKAHEAD = 3  # units of weight-DMA prefetch ahead of compute

        def __init__(self, units):
            # prefetch DMAs fire eagerly at construction so they land on
            # the queue ahead of whatever is emitted before the first pull
            for u in units[: self.LOOKAHEAD]:
                if u.get("dma"):
                    u["dma"]()
            self._gen = self._run(units)
            self.done = False
            self.marks = set()

        def _run(self, units):
            for i, u in enumerate(units):
                nxt = (units[i + self.LOOKAHEAD]
                       if i + self.LOOKAHEAD < len(units) else None)
                if nxt is not None and nxt.get("dma"):
                    nxt["dma"]()
                for cyc in u["comp"]():
                    yield ("step", cyc)
                if u.get("mark") is not None:
                    yield ("mark", u["mark"])

        def _advance(self):
            try:
                item = next(self._gen)
            except StopIteration:
                self.done = True
                return None
            if item[0] == "mark":
                self.marks.add(item[1])
            return item

        def drain_until(self, key):
            while not self.done and key not in self.marks:
                self._advance()

        def pull(self, budget):
            c = 0
            while not self.done and c < budget:
                it = self._advance()
                if it is not None and it[0] == "step":
                    c += it[1]

        def drain(self):
            while not self.done:
                self._advance()

    with tile.TileContext(nc) as tc:
        with (
            tc.tile_pool(name="const", bufs=1) as constp,
            tc.tile_pool(name="kpool", bufs=1) as kpool,
            tc.tile_pool(name="vpool", bufs=1) as vpool,
            tc.tile_pool(name="xpool", bufs=4) as xpool,
            tc.tile_pool(name="qpool", bufs=2) as qpool,
            tc.tile_pool(name="valspool", bufs=2) as valspool,
            tc.tile_pool(name="wpool", bufs=4) as wpool,
            tc.tile_pool(name="owpool", bufs=2) as owpool,
            tc.tile_pool(name="stagep", bufs=2) as stagep,
            tc.tile_pool(name="opartp", bufs=2) as opartp,
            tc.tile_pool(name="ptpool", bufs=4) as ptpool,
            tc.tile_pool(name="opool", bufs=3) as opool,
            tc.tile_pool(name="lgps", bufs=2, space="PSUM") as lgps,
            tc.tile_pool(name="pvps", bufs=2, space="PSUM") as pvps,
            tc.tile_pool(name="mmps", bufs=2, space="PSUM") as mmps,
        ):
            # ---- x chunk t0 first: Q(qi0, c0) only needs x_t0 + wq_c0, so
            #      those DMAs lead the queue; t0 is split in half so the
            #      first projection matmuls start one half-transfer earlier
            xts = []
            xt0 = xpool.tile([P, ND, 512], cdt, tag="x", name="xt0")
            nc.sync.dma_start(xt0[:, 0:ND // 2, :], xT_r[:, 0:ND // 2, ts(0, 512)])
            xts.append(xt0)

            # ---- constants (emitted after x_t0; weight DMAs from the
            #      stream's one-unit lookahead land between these) ----
            ident = constp.tile([P, P], cdt)
            make_identity(nc, ident[:])

            # ---- persistent SBUF tensors ----
            ksb = kpool.tile([P, NPAIR, S], cdt)       # K^T [p, pair, tok]
            vsb = vpool.tile([P, NK, H, hd1], cdt)     # V [tok_p, kc, head, 65]
            nc.vector.memset(vsb[:, :, :, hd:hd1], 1.0)

            qsb = [qpool.tile([P, ND, 512], cdt, tag="q", name=f"qsb{qi}")
                   for qi in range(NQ)]
            valsb = [valspool.tile([P, ND, 512], cdt, tag="vals",
                                   name=f"valsb{qi}") for qi in range(NQ)]

            # ---- projection work units (filler stream A) ----
            def q_unit(qi, c):
                ref = {}

                def dma():
                    wt = wpool.tile([P, ND, P], cdt, tag="w",
                                    name=f"wq{qi}_{c}")
                    nc.sync.dma_start(wt[:], wqT_r[:, :, ts(c, P)])
                    ref["wt"] = wt

                def comp():
                    ps = mmps.tile([P, 512], f32, tag="mm", name=f"qps{qi}{c}")
                    for d in range(ND):
                        mm(ps[:], ref["wt"][:, d, :], xts[qi][:, d, :],
                           d == 0, d == ND - 1)
                        if d % 2 == 1:
                            yield 1024
                    nc.vector.tensor_scalar_add(qsb[qi][:, c, :], ps[:],
                                                bqs[:, c:c + 1])

                return {"dma": dma, "comp": comp}

            def k_unit(c, t, kref):
                def dma():
                    wt = wpool.tile([P, ND, P], cdt, tag="w", name=f"wk{c}")
                    nc.sync.dma_start(wt[:], wkT_r[:, :, ts(c, P)])
                    kref["wt"] = wt

                def comp():
                    ps = mmps.tile([P, 512], f32, tag="mm", name=f"kps{c}{t}")
                    for d in range(ND):
                        mm(ps[:], kref["wt"][:, d, :], xts[t][:, d, :],
                           d == 0, d == ND - 1)
                        if d % 2 == 1:
                            yield 1024
                    nc.vector.tensor_scalar_add(ksb[:, c, ts(t, 512)], ps[:],
                                                bks[:, c:c + 1])

                return {"dma": dma if t == 0 else None, "comp": comp}

            def v_unit(p, u, vref):
                # unit u covers token chunks kc = 4u..4u+3 for heads 2p,2p+1
                def dma():
                    wt = wpool.tile([P, ND, P], cdt, tag="w", name=f"wv{p}")
                    nc.sync.dma_start(wt[:], wvT_r[:, :, ts(p, P)])
                    vref["wt"] = wt

                def comp():
                    ps = mmps.tile([P, 4, P], f32, tag="mm", name=f"vps{p}{u}")
                    for i in range(4):
                        t, s = divmod(4 * u + i, 4)
                        for d in range(ND):
                            # token-major output [tok, dout]: x slice is the
                            # stationary operand.  start/stop once per PSUM
                            # bank: start=True zeroes the whole 2KB
                            # zero-region, so only the first matmul into the
                            # tile may set it
                            mm(ps[:, i, :], xts[t][:, d, ts(s, P)],
                               vref["wt"][:, d, :],
                               i == 0 and d == 0, i == 3 and d == ND - 1)
                            if d % 2 == 1:
                                yield 256
                    bslc = bvb[:, ts(p, P)].rearrange("p (h e) -> p h e", e=hd)
                    for i in range(4):
                        nc.vector.tensor_tensor(
                            vsb[:, 4 * u + i, 2 * p:2 * p + 2, 0:hd],
                            ps[:, i, :].rearrange("p (h e) -> p h e", e=hd),
                            bslc, op=add)

                return {"dma": dma if u == 0 else None, "comp": comp}

            # attention processes the two q-halves interleaved at pair
            # granularity so projection (filler) demand spreads over all
            # iterations instead of front-loading into qi0's pairs
            order = [(0, 0), (0, 1)]
            for p in range(2, NPAIR):
                order += [(0, p), (1, p - 2)]
            order += [(1, NPAIR - 2), (1, NPAIR - 1)]

            units_a = []
            for qi, c in order:
                if qi == 0:
                    units_a.append(q_unit(0, c))
                    kref, vref = {}, {}
                    for t in range(NT):
                        units_a.append(k_unit(c, t, kref))
                    for u in range(NT):
                        units_a.append(v_unit(c, u, vref))
                    units_a[-1]["mark"] = (0, c)
                else:
                    units_a.append(q_unit(1, c))
                    units_a[-1]["mark"] = (1, c)

            # creating the stream queues the first weight DMAs (wq0, wk0)
            # right behind x_t0, before the bulkier x_t1..3 transfers
            A = Stream(units_a)

            nc.sync.dma_start(xt0[:, ND // 2:ND, :],
                              xT_r[:, ND // 2:ND, ts(0, 512)])
            bqs = constp.tile([P, ND], f32)
            nc.sync.dma_start(bqs[:], bq.ap().rearrange("(c p) -> p c", p=P))
            bks = constp.tile([P, ND], f32)
            nc.sync.dma_start(bks[:], bk.ap().rearrange("(c p) -> p c", p=P))
            for t in range(1, NT):
                xt = xpool.tile([P, ND, 512], cdt, tag="x", name=f"xt{t}")
                nc.sync.dma_start(xt[:], xT_r[:, :, ts(t, 512)])
                xts.append(xt)
            bvb = constp.tile([P, D], f32)
            nc.sync.dma_start(bvb[:], bv.ap().unsqueeze(0).to_broadcast((P, D)))
            bob = constp.tile([P, D], f32)
            nc.sync.dma_start(bob[:], bo.ap().unsqueeze(0).to_broadcast((P, D)))

            # ---- o-projection units (stream B): contraction split in two
            #      halves so pairs 0-3's contribution runs as filler right
            #      after slot (qi,3); only half 1 remains after the last pair
            ow_tiles = {}
            opart = [opartp.tile([P, ND, 512], cdt, tag="opart",
                                 name=f"opart{qi}") for qi in range(NQ)]

            def o_unit(qi, s, g, half):
                def dma():
                    if g not in ow_tiles:
                        t = owpool.tile([P, ND, 512], cdt, tag="ow",
                                        name=f"ow{g}")
                        nc.sync.dma_start(t[:], owT_r[:, :, ts(g, 512)])
                        ow_tiles[g] = t

                def comp():
                    nd2 = ND // 2
                    ps = mmps.tile([P, 512], f32, tag="mm",
                                   name=f"ops{qi}{s}{g}{half}")
                    for d in range(half * nd2, half * nd2 + nd2):
                        mm(ps[:], valsb[qi][:, d, ts(s, P)],
                           ow_tiles[g][:, d, :],
                           d == half * nd2, d == half * nd2 + nd2 - 1)
                        if d % 2 == 1:
                            yield 1024
                    if half == 0:
                        # bias folded into the partial
                        nc.vector.tensor_tensor(opart[qi][:, g * 4 + s, :],
                                                ps[:], bob[:, ts(g, 512)],
                                                op=add)
                    else:
                        osb = opool.tile([P, 512], cdt, tag="o",
                                         name=f"o{qi}{s}{g}")
                        nc.vector.tensor_tensor(osb[:], ps[:],
                                                opart[qi][:, g * 4 + s, :],
                                                op=add)
                        nc.sync.dma_start(
                            out.ap()[qi * 512 + s * P: qi * 512 + (s + 1) * P,
                                     ts(g, 512)],
                            osb[:])

                return {"dma": dma, "comp": comp}

            bstreams = []

            def pull_filler(budget=1024):
                if not A.done:
                    A.pull(budget)
                    return
                for b in bstreams:
                    if not b.done:
                        b.pull(budget)
                        return

            # ---- attention: ACT-paced inner loop with PE fillers.  Each
            #      pair's transpose+evac (PE work that waits on the DVE
            #      normalize) is deferred into the NEXT pair's kc=1 slot so
            #      the PE never stalls on the normalize latency. ----
            pending_finalize = [None]

            def run_finalize():
                if pending_finalize[0] is not None:
                    fn = pending_finalize[0]
                    pending_finalize[0] = None
                    fn()

            for qi, p in order:
                    A.drain_until((qi, p))
                    pv = [pvps.tile([P, 4, hd1], f32, tag="pv",
                                    name=f"pv{qi}_{p}_{j}") for j in range(2)]
                    # PV consumes exp output two iterations late so the PE
                    # never waits on the ACT pipeline+semaphore latency
                    LAG = 2
                    pts = {}
                    for kc in range(NK + LAG):
                        if kc < NK:
                            lg = lgps.tile([P, 2, 512], f32, tag="lg")
                            for j in range(2):
                                off = j * hd
                                mm(lg[:, j, :], ksb[off:off + hd, p, ts(kc, P)],
                                   qsb[qi][off:off + hd, p, :], True, True)
                            pt = ptpool.tile([P, 2, 512], cdt, tag="pt")
                            nc.scalar.activation(pt[:], lg[:], Exp, scale=scale)
                            pts[kc] = pt
                        if kc >= LAG:
                            k0 = kc - LAG
                            for j in range(2):
                                for qb in range(4):
                                    # one accumulation group per pv bank:
                                    # start only zeroes once (k0=0, qb=0)
                                    mm(pv[j][:, qb, :],
                                       pts[k0][:, j, ts(qb, P)],
                                       vsb[:, k0, 2 * p + j, :],
                                       k0 == 0 and qb == 0,
                                       k0 == NK - 1 and qb == 3)
                            del pts[k0]
                        if kc == 1:
                            run_finalize()
                        pull_filler(1024)

                    # normalize (per-partition denom) on DVE right away;
                    # the dependent PE transposes + evacs run deferred
                    stgs = []
                    for j in range(2):
                        stg = stagep.tile([P, 4, hd], cdt, tag="stg",
                                          name=f"stg{qi}_{p}_{j}")
                        rcp = stagep.tile([P, 4], f32, tag="rcp",
                                          name=f"rcp{qi}_{p}_{j}")
                        nc.vector.reciprocal(
                            rcp[:], pv[j][:, :, hd:hd1].rearrange(
                                "p a b -> p (a b)"))
                        for qb in range(4):
                            nc.vector.tensor_scalar(
                                out=stg[:, qb, :],
                                in0=pv[j][:, qb, 0:hd],
                                scalar1=rcp[:, qb:qb + 1],
                                scalar2=None,
                                op0=mult)
                        stgs.append(stg)

                    def make_finalize(qi=qi, p=p, stgs=stgs):
                        def finalize():
                            for j in range(2):
                                for h in range(2):
                                    tp = mmps.tile([P, P], cdt, tag="mm",
                                                   name=f"tp{qi}_{p}_{j}_{h}")
                                    nc.tensor.transpose(
                                        tp[:], stgs[j][:, 2 * h:2 * h + 2, :],
                                        ident[:])
                                    for hh in range(2):
                                        qb = 2 * h + hh
                                        nc.vector.tensor_scalar_add(
                                            valsb[qi][j * hd:(j + 1) * hd, p,
                                                      ts(qb, P)],
                                            tp[hh * hd:(hh + 1) * hd, :], 0.0)
                            if p == 3:
                                bstreams.append(Stream([o_unit(qi, s, g, 0)
                                                        for g in range(2)
                                                        for s in range(4)]))
                            elif p == NPAIR - 1:
                                bstreams.append(Stream([o_unit(qi, s, g, 1)
                                                        for g in range(2)
                                                        for s in range(4)]))
                        return finalize

                    pending_finalize[0] = make_finalize()

            run_finalize()

            A.drain()
            for b in bstreams:
                b.drain()

    nc.compile()
    return nc


def _get_nc(S, D, H, SQ, use_bf16=True):
    key = (S, D, H, SQ, use_bf16)
    if key not in _NC_CACHE:
        _NC_CACHE[key] = _build_nc(S, D, H, SQ, use_bf16)
    return _NC_CACHE[key]


def _host_prep_weights(qkv_w, qkv_b, o_w, o_b, H, use_bf16=True):
    """Reorder qkv into head-major q/k/v blocks and pre-transpose."""
    import ml_dtypes
    wdt = ml_dtypes.bfloat16 if use_bf16 else np.float32
    D = o_w.shape[0]
    hd = D // H
    qkv3 = qkv_w.reshape(H, 3, hd, D)
    b3 = qkv_b.reshape(H, 3, hd)
    wqT = np.ascontiguousarray(qkv3[:, 0].reshape(D, D).T.astype(wdt))
    wkT = np.ascontiguousarray(qkv3[:, 1].reshape(D, D).T.astype(wdt))
    wvT = np.ascontiguousarray(qkv3[:, 2].reshape(D, D).T.astype(wdt))
    owT = np.ascontiguousarray(o_w.T.astype(wdt))
    return dict(
        wqT=wqT, wkT=wkT, wvT=wvT, owT=owT,
        bq=np.ascontiguousarray(b3[:, 0].reshape(D)),
        bk=np.ascontiguousarray(b3[:, 1].reshape(D)),
        bv=np.ascontiguousarray(b3[:, 2].reshape(D)),
        bo=np.ascontiguousarray(o_b),
    )


def kernel(x, qkv_w, qkv_b, o_w, o_b, _trace=False):
    from concourse.bass_utils import run_bass_kernel_spmd

    x = np.asarray(x, dtype=np.float32)
    qkv_w = np.asarray(qkv_w, dtype=np.float32)
    qkv_b = np.asarray(qkv_b, dtype=np.float32)
    o_w = np.asarray(o_w, dtype=np.float32)
    o_b = np.asarray(o_b, dtype=np.float32)

    B, S, D = x.shape
    H = 16
    n_cores = 8
    halves = n_cores // B           # 2 query-token halves per batch
    SQ = S // halves                # 1024 query tokens per core

    nc = _get_nc(S, D, H, SQ)
    shared = _host_prep_weights(qkv_w, qkv_b, o_w, o_b, H)

    in_maps = []
    for c in range(n_cores):
        b, half = divmod(c, halves)
        # this core's query tokens first; key/value order is irrelevant
        xp = np.concatenate([x[b, half * SQ:(half + 1) * SQ],
                             np.concatenate([x[b, :half * SQ],
                                             x[b, (half + 1) * SQ:]], axis=0)],
                            axis=0)
        m = dict(shared)
        import ml_dtypes
        m["xT"] = np.ascontiguousarray(xp.T.astype(ml_dtypes.bfloat16))
        in_maps.append(m)

    res = run_bass_kernel_spmd(nc, in_maps, list(range(n_cores)),
                               trace=_trace)

    out = np.empty((B, S, D), dtype=np.float32)
    for c in range(n_cores):
        b, half = divmod(c, halves)
        out[b, half * SQ:(half + 1) * SQ] = np.asarray(
            res.results[c]["out"], dtype=np.float32)
    if _trace:
        return out, res
    return out


# revision 37
# speedup vs baseline: 1.0008x; 1.0008x over previous
"""Distributed MHA kernel for one TRN2 chip (8 NeuronCores), Bass/Tile.

Problem: B=4, S=2048, D=1024, H=16 full multi-head attention
(qkv proj -> scaled dot product softmax attention -> o proj).

Sharding (no collectives): core c handles batch b=c//2 and query-token
half c%2 (1024 query tokens).  Each core recomputes K/V projections for
the full 2048 tokens of its batch (+25% PE work, zero cross-core sync).
The host permutes x[b] so the core's query tokens come first; softmax
over keys is permutation invariant, so K/V token order doesn't matter.

On-chip dataflow (per core), bf16 operands, fp32 PSUM:
  logits^T [ktok, q] = K_h^T.T @ Q_h^T  (contract hd=64, head pairs
      packed at partition offsets 0/64)
  P^T = exp(0.125 * logits^T)           (ACT; logits ~ N(0,1), safe)
  PV in [q, d] layout: out[q128, 65] += P^T[ktok, q128].T @ V_aug[ktok, 65]
      (V_aug has a ones column -> col 64 accumulates the softmax
      denominator per q partition; 65-col matmuls instead of 512-col
      halve the PE cycles of the PV stage)
  normalize: per-partition reciprocal+multiply (DVE)
  vals arrives [q, d]-transposed for o-proj -> PE transpose (identity
      matmul) back to [d, q], evacuate to SBUF
  o proj: out[tok, e] = vals^T.T @ o_w^T, contraction split in two
      halves so the first half runs mid-attention; bf16 output store

Scheduling (the whole point): the attention inner loop alone is
ACT(exp)-paced at ~1038ns/iter vs ~640ns of PE work, so all K/V/Q/O
projection matmuls are fed through pull-driven "filler" streams
interleaved between attention matmuls; the PE stays ~95% busy and is
the binding engine end to end.  Stream markers guarantee V/K/Q of
pair p are emitted before attention of pair p; the two q-halves'
attention is interleaved at pair granularity to spread filler demand;
PV consumes exp output with a two-iteration lag to hide the ACT
pipeline+semaphore latency; each pair's transpose+evac is deferred
into the next pair's second iteration so the PE never waits on the
DVE normalize.
"""

import numpy as np

_NC_CACHE = {}


def _build_nc(S, D, H, SQ, use_bf16=True):
    import concourse.bass as bass
    import concourse.mybir as mybir
    import concourse.tile as tile
    from concourse import bacc
    from concourse.bass import ts
    from concourse.masks import make_identity

    f32 = mybir.dt.float32
    cdt = mybir.dt.bfloat16 if use_bf16 else f32
    Exp = mybir.ActivationFunctionType.Exp
    add = mybir.AluOpType.add
    mult = mybir.AluOpType.mult

    P = 128
    hd = D // H            # 64 head dim
    hd1 = hd + 1           # 65: V block + ones column
    ND = D // P            # 8 din/dout chunks
    NT = S // 512          # 4 tok512 chunks
    NQ = SQ // 512         # 2 q512 chunks
    NK = S // P            # 16 k-token chunks
    NPAIR = H // 2         # 8 head pairs (= dout chunks of K/Q)
    scale = 1.0 / float(np.sqrt(hd))

    nc = bacc.Bacc(trn_type="TRN2", debug=False)

    xT = nc.declare_dram_parameter("xT", [D, S], cdt, isOutput=False)
    wqT = nc.declare_dram_parameter("wqT", [D, D], cdt, isOutput=False)
    wkT = nc.declare_dram_parameter("wkT", [D, D], cdt, isOutput=False)
    wvT = nc.declare_dram_parameter("wvT", [D, D], cdt, isOutput=False)
    owT = nc.declare_dram_parameter("owT", [D, D], cdt, isOutput=False)
    bq = nc.declare_dram_parameter("bq", [D], f32, isOutput=False)
    bk = nc.declare_dram_parameter("bk", [D], f32, isOutput=False)
    bv = nc.declare_dram_parameter("bv", [D], f32, isOutput=False)
    bo = nc.declare_dram_parameter("bo", [D], f32, isOutput=False)
    # bf16 output halves the store traffic on the serialized DMA device
    # (host converts back to f32); costs ~0.2% relative error
    out = nc.declare_dram_parameter("out", [SQ, D], cdt, isOutput=True)

    xT_r = xT.ap().rearrange("(c p) s -> p c s", p=P)
    wqT_r = wqT.ap().rearrange("(c p) e -> p c e", p=P)
    wkT_r = wkT.ap().rearrange("(c p) e -> p c e", p=P)
    wvT_r = wvT.ap().rearrange("(c p) e -> p c e", p=P)
    owT_r = owT.ap().rearrange("(c p) e -> p c e", p=P)

    def mm(ps, lhsT, rhs, start, stop):
        nc.tensor.matmul(ps, lhsT, rhs, start=start, stop=stop)

    class Stream:
        """Pull-driven instruction-emission stream with markers.

        Units are dicts {dma: fn|None, comp: genfn, mark: key|None}; the
        comp generator yields approximate PE-cycle counts after each
        chunk of emitted instructions.  DMAs are emitted one unit ahead
        of compute so weights prefetch.
        """

        LOOKAHEAD = 3  # units of weight-DMA prefetch ahead of compute

        def __init__(self, units):
            # prefetch DMAs fire eagerly at construction so they land on
            # the queue ahead of whatever is emitted before the first pull
            for u in units[: self.LOOKAHEAD]:
                if u.get("dma"):
                    u["dma"]()
            self._gen = self._run(units)
            self.done = False
            self.marks = set()

        def _run(self, units):
            for i, u in enumerate(units):
                nxt = (units[i + self.LOOKAHEAD]
                       if i + self.LOOKAHEAD < len(units) else None)
                if nxt is not None and nxt.get("dma"):
                    nxt["dma"]()
                for cyc in u["comp"]():
                    yield ("step", cyc)
                if u.get("mark") is not None:
                    yield ("mark", u["mark"])

        def _advance(self):
            try:
                item = next(self._gen)
            except StopIteration:
                self.done = True
                return None
            if item[0] == "mark":
                self.marks.add(item[1])
            return item

        def drain_until(self, key):
            while not self.done and key not in self.marks:
                self._advance()

        def pull(self, budget):
            c = 0
            while not self.done and c < budget:
                it = self._advance()
                if it is not None and it[0] == "step":
                    c += it[1]

        def drain(self):
            while not self.done:
                self._advance()

    with tile.TileContext(nc) as tc:
        with (
            tc.tile_pool(name="const", bufs=1) as constp,
            tc.tile_pool(name="kpool", bufs=1) as kpool,
            tc.tile_pool(name="vpool", bufs=1) as vpool,
            tc.tile_pool(name="xpool", bufs=4) as xpool,
            tc.tile_pool(name="qpool", bufs=2) as qpool,
            tc.tile_pool(name="valspool", bufs=2) as valspool,
            tc.tile_pool(name="wpool", bufs=4) as wpool,
            tc.tile_pool(name="owpool", bufs=2) as owpool,
            tc.tile_pool(name="stagep", bufs=2) as stagep,
            tc.tile_pool(name="opartp", bufs=2) as opartp,
            tc.tile_pool(name="ptpool", bufs=4) as ptpool,
            tc.tile_pool(name="opool", bufs=3) as opool,
            tc.tile_pool(name="lgps", bufs=2, space="PSUM") as lgps,
            tc.tile_pool(name="pvps", bufs=2, space="PSUM") as pvps,
            tc.tile_pool(name="mmps", bufs=2, space="PSUM") as mmps,
        ):
            # ---- x chunk t0 first: Q(qi0, c0) only needs x_t0 + wq_c0, so
            #      those DMAs lead the queue; t0 is split in half so the
            #      first projection matmuls start one half-transfer earlier
            xts = []
            xt0 = xpool.tile([P, ND, 512], cdt, tag="x", name="xt0")
            nc.sync.dma_start(xt0[:, 0:2, :], xT_r[:, 0:2, ts(0, 512)])
            xts.append(xt0)

            # ---- constants (emitted after x_t0; weight DMAs from the
            #      stream's one-unit lookahead land between these) ----
            ident = constp.tile([P, P], cdt)
            make_identity(nc, ident[:])

            # ---- persistent SBUF tensors ----
            ksb = kpool.tile([P, NPAIR, S], cdt)       # K^T [p, pair, tok]
            vsb = vpool.tile([P, NK, H, hd1], cdt)     # V [tok_p, kc, head, 65]
            nc.vector.memset(vsb[:, :, :, hd:hd1], 1.0)

            qsb = [qpool.tile([P, ND, 512], cdt, tag="q", name=f"qsb{qi}")
                   for qi in range(NQ)]
            valsb = [valspool.tile([P, ND, 512], cdt, tag="vals",
                                   name=f"valsb{qi}") for qi in range(NQ)]

            # ---- projection work units (filler stream A) ----
            def q_unit(qi, c):
                ref = {}

                def dma():
                    wt = wpool.tile([P, ND, P], cdt, tag="w",
                                    name=f"wq{qi}_{c}")
                    nc.sync.dma_start(wt[:], wqT_r[:, :, ts(c, P)])
                    ref["wt"] = wt

                def comp():
                    ps = mmps.tile([P, 512], f32, tag="mm", name=f"qps{qi}{c}")
                    for d in range(ND):
                        mm(ps[:], ref["wt"][:, d, :], xts[qi][:, d, :],
                           d == 0, d == ND - 1)
                        if d % 2 == 1:
                            yield 1024
                    nc.vector.tensor_scalar_add(qsb[qi][:, c, :], ps[:],
                                                bqs[:, c:c + 1])

                return {"dma": dma, "comp": comp}

            def k_unit(c, t, kref):
                def dma():
                    wt = wpool.tile([P, ND, P], cdt, tag="w", name=f"wk{c}")
                    nc.sync.dma_start(wt[:], wkT_r[:, :, ts(c, P)])
                    kref["wt"] = wt

                def comp():
                    ps = mmps.tile([P, 512], f32, tag="mm", name=f"kps{c}{t}")
                    for d in range(ND):
                        mm(ps[:], kref["wt"][:, d, :], xts[t][:, d, :],
                           d == 0, d == ND - 1)
                        if d % 2 == 1:
                            yield 1024
                    nc.vector.tensor_scalar_add(ksb[:, c, ts(t, 512)], ps[:],
                                                bks[:, c:c + 1])

                return {"dma": dma if t == 0 else None, "comp": comp}

            def v_unit(p, u, vref):
                # unit u covers token chunks kc = 4u..4u+3 for heads 2p,2p+1
                def dma():
                    wt = wpool.tile([P, ND, P], cdt, tag="w", name=f"wv{p}")
                    nc.sync.dma_start(wt[:], wvT_r[:, :, ts(p, P)])
                    vref["wt"] = wt

                def comp():
                    ps = mmps.tile([P, 4, P], f32, tag="mm", name=f"vps{p}{u}")
                    for i in range(4):
                        t, s = divmod(4 * u + i, 4)
                        for d in range(ND):
                            # token-major output [tok, dout]: x slice is the
                            # stationary operand.  start/stop once per PSUM
                            # bank: start=True zeroes the whole 2KB
                            # zero-region, so only the first matmul into the
                            # tile may set it
                            mm(ps[:, i, :], xts[t][:, d, ts(s, P)],
                               vref["wt"][:, d, :],
                               i == 0 and d == 0, i == 3 and d == ND - 1)
                            if d % 2 == 1:
                                yield 256
                    bslc = bvb[:, ts(p, P)].rearrange("p (h e) -> p h e", e=hd)
                    for i in range(4):
                        nc.vector.tensor_tensor(
                            vsb[:, 4 * u + i, 2 * p:2 * p + 2, 0:hd],
                            ps[:, i, :].rearrange("p (h e) -> p h e", e=hd),
                            bslc, op=add)

                return {"dma": dma if u == 0 else None, "comp": comp}

            # attention processes the two q-halves interleaved at pair
            # granularity so projection (filler) demand spreads over all
            # iterations instead of front-loading into qi0's pairs
            order = [(0, 0), (0, 1)]
            for p in range(2, NPAIR):
                order += [(0, p), (1, p - 2)]
            order += [(1, NPAIR - 2), (1, NPAIR - 1)]

            units_a = []
            for qi, c in order:
                if qi == 0:
                    units_a.append(q_unit(0, c))
                    kref, vref = {}, {}
                    for t in range(NT):
                        units_a.append(k_unit(c, t, kref))
                    for u in range(NT):
                        units_a.append(v_unit(c, u, vref))
                    units_a[-1]["mark"] = (0, c)
                else:
                    units_a.append(q_unit(1, c))
                    units_a[-1]["mark"] = (1, c)

            # creating the stream queues the first weight DMAs (wq0, wk0)
            # right behind x_t0, before the bulkier x_t1..3 transfers
            A = Stream(units_a)

            nc.sync.dma_start(xt0[:, 2:4, :], xT_r[:, 2:4, ts(0, 512)])
            nc.sync.dma_start(xt0[:, ND // 2:ND, :],
                              xT_r[:, ND // 2:ND, ts(0, 512)])
            bqs = constp.tile([P, ND], f32)
            nc.sync.dma_start(bqs[:], bq.ap().rearrange("(c p) -> p c", p=P))
            bks = constp.tile([P, ND], f32)
            nc.sync.dma_start(bks[:], bk.ap().rearrange("(c p) -> p c", p=P))
            for t in range(1, NT):
                xt = xpool.tile([P, ND, 512], cdt, tag="x", name=f"xt{t}")
                nc.sync.dma_start(xt[:], xT_r[:, :, ts(t, 512)])
                xts.append(xt)
            bvb = constp.tile([P, D], f32)
            nc.sync.dma_start(bvb[:], bv.ap().unsqueeze(0).to_broadcast((P, D)))
            bob = constp.tile([P, D], f32)
            nc.sync.dma_start(bob[:], bo.ap().unsqueeze(0).to_broadcast((P, D)))

            # ---- o-projection units (stream B): contraction split in two
            #      halves so pairs 0-3's contribution runs as filler right
            #      after slot (qi,3); only half 1 remains after the last pair
            ow_tiles = {}
            opart = [opartp.tile([P, ND, 512], cdt, tag="opart",
                                 name=f"opart{qi}") for qi in range(NQ)]

            def o_unit(qi, s, g, half):
                def dma():
                    if g not in ow_tiles:
                        t = owpool.tile([P, ND, 512], cdt, tag="ow",
                                        name=f"ow{g}")
                        nc.sync.dma_start(t[:], owT_r[:, :, ts(g, 512)])
                        ow_tiles[g] = t

                def comp():
                    # split 6+2: half 0 (pairs 0-5) runs mid-attention, so
                    # only a 2-matmul-deep tail remains after the last pair
                    dr = range(0, 6) if half == 0 else range(6, ND)
                    ps = mmps.tile([P, 512], f32, tag="mm",
                                   name=f"ops{qi}{s}{g}{half}")
                    for d in dr:
                        mm(ps[:], valsb[qi][:, d, ts(s, P)],
                           ow_tiles[g][:, d, :],
                           d == dr[0], d == dr[-1])
                        if d % 2 == 1:
                            yield 1024
                    if half == 0:
                        # bias folded into the partial
                        nc.vector.tensor_tensor(opart[qi][:, g * 4 + s, :],
                                                ps[:], bob[:, ts(g, 512)],
                                                op=add)
                    else:
                        osb = opool.tile([P, 512], cdt, tag="o",
                                         name=f"o{qi}{s}{g}")
                        nc.vector.tensor_tensor(osb[:], ps[:],
                                                opart[qi][:, g * 4 + s, :],
                                                op=add)
                        nc.sync.dma_start(
                            out.ap()[qi * 512 + s * P: qi * 512 + (s + 1) * P,
                                     ts(g, 512)],
                            osb[:])

                return {"dma": dma, "comp": comp}

            bstreams = []

            def pull_filler(budget=1024):
                if not A.done:
                    A.pull(budget)
                    return
                for b in bstreams:
                    if not b.done:
                        b.pull(budget)
                        return

            # ---- attention: ACT-paced inner loop with PE fillers.  Each
            #      pair's transpose+evac (PE work that waits on the DVE
            #      normalize) is deferred into the NEXT pair's kc=1 slot so
            #      the PE never stalls on the normalize latency. ----
            pending_finalize = [None]

            def run_finalize():
                if pending_finalize[0] is not None:
                    fn = pending_finalize[0]
                    pending_finalize[0] = None
                    fn()

            for qi, p in order:
                    A.drain_until((qi, p))
                    pv = [pvps.tile([P, 4, hd1], f32, tag="pv",
                                    name=f"pv{qi}_{p}_{j}") for j in range(2)]
                    # PV consumes exp output two iterations late so the PE
                    # never waits on the ACT pipeline+semaphore latency
                    LAG = 2
                    pts = {}
                    for kc in range(NK + LAG):
                        if kc < NK:
                            lg = lgps.tile([P, 2, 512], f32, tag="lg")
                            for j in range(2):
                                off = j * hd
                                mm(lg[:, j, :], ksb[off:off + hd, p, ts(kc, P)],
                                   qsb[qi][off:off + hd, p, :], True, True)
                            pt = ptpool.tile([P, 2, 512], cdt, tag="pt")
                            nc.scalar.activation(pt[:], lg[:], Exp, scale=scale)
                            pts[kc] = pt
                        if kc >= LAG:
                            k0 = kc - LAG
                            for j in range(2):
                                for qb in range(4):
                                    # one accumulation group per pv bank:
                                    # start only zeroes once (k0=0, qb=0)
                                    mm(pv[j][:, qb, :],
                                       pts[k0][:, j, ts(qb, P)],
                                       vsb[:, k0, 2 * p + j, :],
                                       k0 == 0 and qb == 0,
                                       k0 == NK - 1 and qb == 3)
                            del pts[k0]
                        if kc == 1:
                            run_finalize()
                        pull_filler(1024)

                    # normalize (per-partition denom) on DVE right away;
                    # the dependent PE transposes + evacs run deferred
                    stgs = []
                    for j in range(2):
                        stg = stagep.tile([P, 4, hd], cdt, tag="stg",
                                          name=f"stg{qi}_{p}_{j}")
                        rcp = stagep.tile([P, 4], f32, tag="rcp",
                                          name=f"rcp{qi}_{p}_{j}")
                        nc.vector.reciprocal(
                            rcp[:], pv[j][:, :, hd:hd1].rearrange(
                                "p a b -> p (a b)"))
                        for qb in range(4):
                            nc.vector.tensor_scalar(
                                out=stg[:, qb, :],
                                in0=pv[j][:, qb, 0:hd],
                                scalar1=rcp[:, qb:qb + 1],
                                scalar2=None,
                                op0=mult)
                        stgs.append(stg)

                    def make_finalize(qi=qi, p=p, stgs=stgs):
                        def finalize():
                            for j in range(2):
                                for h in range(2):
                                    tp = mmps.tile([P, P], cdt, tag="mm",
                                                   name=f"tp{qi}_{p}_{j}_{h}")
                                    nc.tensor.transpose(
                                        tp[:], stgs[j][:, 2 * h:2 * h + 2, :],
                                        ident[:])
                                    for hh in range(2):
                                        qb = 2 * h + hh
                                        nc.vector.tensor_scalar_add(
                                            valsb[qi][j * hd:(j + 1) * hd, p,
                                                      ts(qb, P)],
                                            tp[hh * hd:(hh + 1) * hd, :], 0.0)
                            if p == 5:
                                bstreams.append(Stream([o_unit(qi, s, g, 0)
                                                        for g in range(2)
                                                        for s in range(4)]))
                            elif p == NPAIR - 1:
                                bstreams.append(Stream([o_unit(qi, s, g, 1)
                                                        for g in range(2)
                                                        for s in range(4)]))
                        return finalize

                    pending_finalize[0] = make_finalize()

            run_finalize()

            A.drain()
            for b in bstreams:
                b.drain()

    nc.compile()
    return nc


def _get_nc(S, D, H, SQ, use_bf16=True):
    key = (S, D, H, SQ, use_bf16)
    if key not in _NC_CACHE:
        _NC_CACHE[key] = _build_nc(S, D, H, SQ, use_bf16)
    return _NC_CACHE[key]


def _host_prep_weights(qkv_w, qkv_b, o_w, o_b, H, use_bf16=True):
    """Reorder qkv into head-major q/k/v blocks and pre-transpose."""
    import ml_dtypes
    wdt = ml_dtypes.bfloat16 if use_bf16 else np.float32
    D = o_w.shape[0]
    hd = D // H
    qkv3 = qkv_w.reshape(H, 3, hd, D)
    b3 = qkv_b.reshape(H, 3, hd)
    wqT = np.ascontiguousarray(qkv3[:, 0].reshape(D, D).T.astype(wdt))
    wkT = np.ascontiguousarray(qkv3[:, 1].reshape(D, D).T.astype(wdt))
    wvT = np.ascontiguousarray(qkv3[:, 2].reshape(D, D).T.astype(wdt))
    owT = np.ascontiguousarray(o_w.T.astype(wdt))
    return dict(
        wqT=wqT, wkT=wkT, wvT=wvT, owT=owT,
        bq=np.ascontiguousarray(b3[:, 0].reshape(D)),
        bk=np.ascontiguousarray(b3[:, 1].reshape(D)),
        bv=np.ascontiguousarray(b3[:, 2].reshape(D)),
        bo=np.ascontiguousarray(o_b),
    )


def kernel(x, qkv_w, qkv_b, o_w, o_b, _trace=False):
    from concourse.bass_utils import run_bass_kernel_spmd

    x = np.asarray(x, dtype=np.float32)
    qkv_w = np.asarray(qkv_w, dtype=np.float32)
    qkv_b = np.asarray(qkv_b, dtype=np.float32)
    o_w = np.asarray(o_w, dtype=np.float32)
    o_b = np.asarray(o_b, dtype=np.float32)

    B, S, D = x.shape
    H = 16
    n_cores = 8
    halves = n_cores // B           # 2 query-token halves per batch
    SQ = S // halves                # 1024 query tokens per core

    nc = _get_nc(S, D, H, SQ)
    shared = _host_prep_weights(qkv_w, qkv_b, o_w, o_b, H)

    in_maps = []
    for c in range(n_cores):
        b, half = divmod(c, halves)
        # this core's query tokens first; key/value order is irrelevant
        xp = np.concatenate([x[b, half * SQ:(half + 1) * SQ],
                             np.concatenate([x[b, :half * SQ],
                                             x[b, (half + 1) * SQ:]], axis=0)],
                            axis=0)
        m = dict(shared)
        import ml_dtypes
        m["xT"] = np.ascontiguousarray(xp.T.astype(ml_dtypes.bfloat16))
        in_maps.append(m)

    res = run_bass_kernel_spmd(nc, in_maps, list(range(n_cores)),
                               trace=_trace)

    out = np.empty((B, S, D), dtype=np.float32)
    for c in range(n_cores):
        b, half = divmod(c, halves)
        out[b, half * SQ:(half + 1) * SQ] = np.asarray(
            res.results[c]["out"], dtype=np.float32)
    if _trace:
        return out, res
    return out
